# revision 9
# baseline (speedup 1.0000x reference)
"""Causal self-attention (single head, S=4096, D=1024) on 8 TRN2 NeuronCores.

Strategy (sequence-parallel, instruction-count-optimized):
  - Core c owns contiguous query rows [512c, 512(c+1)) and computes the
    K/V projections for the same rows; K^T/V are AllGathered in bf16.
  - Scores are computed TRANSPOSED (S^T[j, i], keys on partitions): softmax
    needs no max-subtraction (scores are O(5)), exp(S^T) feeds the PV matmul
    directly as lhsT (no transposes), row sums come from a ones-vector matmul
    accumulated in PSUM across the whole loop.
  - Every core runs the same fully-static program over all 8 key windows
    (window W = AG rank block W); causality and the padded (future) windows
    are handled by per-core multiplicative mask streams prepared on the host.
"""

import numpy as np
import ml_dtypes

S = 4096
D = 1024
N_CORES = 8
P = 128
SH = 512              # per-core query shard rows / key window
N_WIN = 8
KT_ELEMS = D * SH           # per-rank kT block elements in AG buffer
V_ELEMS = SH * D
RANK_ELEMS = KT_ELEMS + V_ELEMS
AG_OUT_ELEMS = N_CORES * RANK_ELEMS
SCALE = 1.0 / 32.0          # 1/sqrt(D)

_CACHE = {}


def _build(win_mult=1):
    import concourse.bass as bass
    import concourse.mybir as mybir
    import concourse.tile as tile
    from concourse import bacc

    bf16 = mybir.dt.bfloat16
    f32 = mybir.dt.float32

    nc = bacc.Bacc("TRN2", target_bir_lowering=False, debug=False,
                   num_devices=N_CORES)

    # ---- per-core I/O ----
    wq = nc.dram_tensor("wq", [P, 8, D], bf16, kind="ExternalInput")
    wk = nc.dram_tensor("wk", [P, 8, D], bf16, kind="ExternalInput")
    wv = nc.dram_tensor("wv", [P, 8, D], bf16, kind="ExternalInput")
    xs = nc.dram_tensor("xs", [P, 8, SH], bf16, kind="ExternalInput")  # x^T shard
    maskd = nc.dram_tensor("mask", [N_WIN, 2, P, 2, SH], bf16, kind="ExternalInput")
    onesd = nc.dram_tensor("ones", [P, 1], bf16, kind="ExternalInput")
    outd = nc.dram_tensor("out", [SH, D], f32, kind="ExternalOutput")

    agin = nc.dram_tensor("agin", [1, RANK_ELEMS], bf16)
    agout = nc.dram_tensor("agout", [1, AG_OUT_ELEMS], bf16, addr_space="Shared")
    rs_dram = nc.dram_tensor("rs_dram", [1, SH], f32)

    with tile.TileContext(nc) as tc:
        with tc.tile_pool(name="wpool", bufs=3) as wpool, \
             tc.tile_pool(name="xpool", bufs=1) as xpool, \
             tc.tile_pool(name="qt", bufs=1) as qtpool, \
             tc.tile_pool(name="stage", bufs=3) as stage, \
             tc.tile_pool(name="consts", bufs=1) as consts, \
             tc.tile_pool(name="accs", bufs=1) as accs:

            # ---------------- Phase 1: projections ----------------
            xs_sb = xpool.tile([P, 8, SH], bf16, name="xs_sb")
            nc.sync.dma_start(xs_sb[:], xs[:])
            wk_sb = wpool.tile([P, 8, D], bf16, name="wk_sb", tag="w")
            nc.sync.dma_start(wk_sb[:], wk[:])
            wv_sb = wpool.tile([P, 8, D], bf16, name="wv_sb", tag="w")
            nc.sync.dma_start(wv_sb[:], wv[:])
            wq_sb = wpool.tile([P, 8, D], bf16, name="wq_sb", tag="w")
            nc.sync.dma_start(wq_sb[:], wq[:])

            with tc.tile_pool(name="pps", bufs=2, space="PSUM") as pps, \
                 tc.tile_pool(name="ppsv", bufs=2, space="PSUM") as ppsv:
                # kT_c: [8 dko][128 dp][512 j] into agin[0 : KT_ELEMS]
                for dt2 in range(4):          # two d-tiles per psum tile
                    ps = pps.tile([P, 2, SH], f32, name=f"kt_ps{dt2}", tag="ktps")
                    for h in range(2):
                        for ko in range(8):
                            d0 = (dt2 * 2 + h) * P
                            nc.tensor.matmul(ps[:, h, :],
                                             wk_sb[:, ko, d0:d0 + P],
                                             xs_sb[:, ko, :],
                                             start=(ko == 0), stop=(ko == 7))
                    st = stage.tile([P, 2, SH], bf16, name=f"kt_st{dt2}", tag="ktst")
                    nc.vector.tensor_copy(st[:], ps[:])
                    dst = bass.AP(agin, dt2 * (2 * P * SH),
                                  [[SH, P], [P * SH, 2], [1, SH]])
                    nc.sync.dma_start(dst, st[:])

                # v_c: [512 s][1024 d] into agin[KT_ELEMS : ]
                for st_i in range(4):
                    ps = ppsv.tile([P, D], f32, name=f"v_ps{st_i}", tag="vps")
                    for ko in range(8):
                        for dh in range(2):
                            nc.tensor.matmul(
                                ps[:, dh * 512:(dh + 1) * 512],
                                xs_sb[:, ko, st_i * P:(st_i + 1) * P],
                                wv_sb[:, ko, dh * 512:(dh + 1) * 512],
                                start=(ko == 0), stop=(ko == 7))
                    st = stage.tile([P, D], bf16, name=f"v_st{st_i}", tag="vst")
                    nc.vector.tensor_copy(st[:], ps[:])
                    dst = bass.AP(agin, KT_ELEMS + st_i * (P * D),
                                  [[D, P], [1, D]])
                    nc.sync.dma_start(dst, st[:])

                # ---------------- Phase 2: AllGather K/V ----------------
                nc.gpsimd.collective_compute(
                    "AllGather", mybir.AluOpType.bypass,
                    replica_groups=[list(range(N_CORES))],
                    ins=[agin.ap().opt()],
                    outs=[agout.ap().opt()],
                )

                # qT_c: keep in SBUF [128 dp, 8 dko, 512 i] (overlaps AG)
                qt_sb = qtpool.tile([P, 8, SH], bf16, name="qt_sb")
                for dt2 in range(4):
                    ps = pps.tile([P, 2, SH], f32, name=f"q_ps{dt2}", tag="ktps")
                    for h in range(2):
                        for ko in range(8):
                            d0 = (dt2 * 2 + h) * P
                            nc.tensor.matmul(ps[:, h, :],
                                             wq_sb[:, ko, d0:d0 + P],
                                             xs_sb[:, ko, :],
                                             start=(ko == 0), stop=(ko == 7))
                    nc.vector.tensor_copy(qt_sb[:, 2 * dt2:2 * dt2 + 2, :], ps[:])

            # ---------------- Phase 3: attention ----------------
            ones_sb = consts.tile([P, 1], bf16, name="ones_sb")
            nc.sync.dma_start(ones_sb[:], onesd[:])
            acc_out = accs.tile([P, 4, D], f32, name="acc_out")
            nc.vector.memset(acc_out[:], 0.0)

            with tc.tile_pool(name="ktw", bufs=2) as ktw, \
                 tc.tile_pool(name="vw", bufs=2) as vw, \
                 tc.tile_pool(name="mk", bufs=2) as mkp, \
                 tc.tile_pool(name="pt", bufs=3) as ptp, \
                 tc.tile_pool(name="stps", bufs=1, space="PSUM") as stps, \
                 tc.tile_pool(name="pvps", bufs=1, space="PSUM") as pvps, \
                 tc.tile_pool(name="rsps", bufs=1, space="PSUM") as rsps:

                rs_ps = rsps.tile([1, SH], f32, name="rs_ps")

                n_win_total = N_WIN * win_mult
                for wi in range(n_win_total):
                    W = wi % N_WIN
                    first = wi == 0
                    last = wi == n_win_total - 1
                    kt_w = ktw.tile([P, 8, SH], bf16, name=f"kt_w{wi}", tag="ktw")
                    nc.sync.dma_start(
                        kt_w[:], bass.AP(agout, W * RANK_ELEMS,
                                         [[SH, P], [P * SH, 8], [1, SH]]))
                    v_w = vw.tile([P, 4, D], bf16, name=f"v_w{wi}", tag="vw")
                    nc.sync.dma_start(
                        v_w[:], bass.AP(agout, W * RANK_ELEMS + KT_ELEMS,
                                        [[D, P], [P * D, 4], [1, D]]))

                    pv_ps = pvps.tile([P, 2, D], f32, name=f"pv{wi}", tag="pv")
                    pts = []
                    for pair in range(2):
                        st_ps = stps.tile([P, 2, SH], f32,
                                          name=f"st{wi}_{pair}", tag="st")
                        for js in range(2):
                            jj = pair * 2 + js
                            for ko in range(8):
                                nc.tensor.matmul(
                                    st_ps[:, js, :],
                                    kt_w[:, ko, jj * P:(jj + 1) * P],
                                    qt_sb[:, ko, :],
                                    start=(ko == 0), stop=(ko == 7))
                        pt = ptp.tile([P, 2, SH], bf16, name=f"pt{wi}_{pair}",
                                      tag="pt")
                        nc.scalar.activation(pt[:], st_ps[:],
                                             mybir.ActivationFunctionType.Exp,
                                             scale=SCALE)
                        mk = mkp.tile([P, 2, SH], bf16, name=f"mk{wi}_{pair}",
                                      tag="mk")
                        nc.sync.dma_start(mk[:], maskd[W, pair])
                        nc.vector.tensor_mul(pt[:], pt[:], mk[:])
                        pts.append(pt)
                        for js in range(2):
                            nc.tensor.matmul(
                                rs_ps[:], ones_sb[:], pt[:, js, :],
                                start=(first and pair == 0 and js == 0),
                                stop=(last and pair == 1 and js == 1))

                    # PV: two isub-pair passes over all 4 j-subtiles
                    for half in range(2):
                        for pair in range(2):
                            for js in range(2):
                                for ib in range(2):
                                    iblk = half * 2 + ib
                                    for dh in range(2):
                                        nc.tensor.matmul(
                                            pv_ps[:, ib, dh * 512:(dh + 1) * 512],
                                            pts[pair][:, js,
                                                      iblk * P:(iblk + 1) * P],
                                            v_w[:, pair * 2 + js,
                                                dh * 512:(dh + 1) * 512],
                                            start=(pair == 0 and js == 0),
                                            stop=(pair == 1 and js == 1))
                        for ib in range(2):
                            iblk = half * 2 + ib
                            nc.vector.tensor_add(acc_out[:, iblk, :],
                                                 acc_out[:, iblk, :],
                                                 pv_ps[:, ib, :])

                # ---------------- finalize: divide by row sums ----------------
                rs_sb = consts.tile([1, SH], f32, name="rs_sb")
                nc.vector.reciprocal(rs_sb[:], rs_ps[:])
                nc.sync.dma_start(rs_dram.ap(), rs_sb[:])
                recipT = consts.tile([P, 4], f32, name="recipT")
                nc.sync.dma_start(
                    recipT[:],
                    rs_dram.ap().rearrange("o (ib p) -> (o p) ib", p=P))
                o_f32 = accs.tile([P, 4, D], f32, name="o_f32")
                nc.vector.tensor_tensor(
                    o_f32[:], acc_out[:],
                    recipT[:, :, None].to_broadcast((P, 4, D)),
                    mybir.AluOpType.mult)
                nc.sync.dma_start(
                    outd.ap().rearrange("(ib p) d -> p ib d", p=P), o_f32[:])

    nc.compile()
    return nc


def _host_inputs(x, W_query, W_key, W_value):
    bf = ml_dtypes.bfloat16

    def wprep(W):
        return np.ascontiguousarray(
            W.reshape(8, P, D).transpose(1, 0, 2)).astype(bf)

    wq_n, wk_n, wv_n = wprep(W_query), wprep(W_key), wprep(W_value)

    in_maps = []
    for c in range(N_CORES):
        rows = np.arange(SH * c, SH * (c + 1))
        xt = x[rows].T.reshape(8, P, SH).transpose(1, 0, 2)
        xs_n = np.ascontiguousarray(xt).astype(bf)

        # mask[W, pair, p, js, i]: valid iff key (512W + (2*pair+js)*128 + p)
        #                               <= query (512c + i)
        mask = np.zeros((N_WIN, 2, P, 2, SH), dtype=np.float32)
        for Wn in range(N_WIN):
            if Wn < c:
                mask[Wn] = 1.0
            elif Wn == c:
                for pair in range(2):
                    for js in range(2):
                        jj = pair * 2 + js
                        j_rel = jj * P + np.arange(P)[:, None]
                        i_rel = np.arange(SH)[None, :]
                        mask[Wn, pair, :, js, :] = (j_rel <= i_rel)
        in_maps.append({
            "wq": wq_n, "wk": wk_n, "wv": wv_n, "xs": xs_n,
            "mask": mask.astype(bf),
            "ones": np.ones((P, 1), dtype=bf),
        })
    return in_maps


def kernel(x, W_query, W_key, W_value):
    from concourse.bass_utils import run_bass_kernel_spmd

    x = np.asarray(x, dtype=np.float32)
    W_query = np.asarray(W_query, dtype=np.float32)
    W_key = np.asarray(W_key, dtype=np.float32)
    W_value = np.asarray(W_value, dtype=np.float32)

    if "nc" not in _CACHE:
        _CACHE["nc"] = _build()
    nc = _CACHE["nc"]

    in_maps = _host_inputs(x, W_query, W_key, W_value)
    res = run_bass_kernel_spmd(nc, in_maps, core_ids=list(range(N_CORES)))

    out = np.empty((S, D), dtype=np.float32)
    for c in range(N_CORES):
        out[SH * c:SH * (c + 1)] = res.results[c]["out"]
    return out


# revision 11
# speedup vs baseline: 8424.9583x; 8424.9583x over previous
"""Causal self-attention (single head, S=4096, D=1024) on 8 TRN2 NeuronCores.

Strategy (sequence-parallel, instruction-count-optimized):
  - Core c owns contiguous query rows [512c, 512(c+1)) and computes the
    K/V projections for the same rows; K^T/V are AllGathered in bf16.
  - Scores are computed TRANSPOSED (S^T[j, i], keys on partitions): softmax
    needs no max-subtraction (scores are O(5)), exp(S^T) feeds the PV matmul
    directly as lhsT (no transposes), row sums come from a ones-vector matmul
    accumulated in PSUM across the whole loop.
  - Every core runs the same fully-static program over all 8 key windows
    (window W = AG rank block W); causality and the padded (future) windows
    are handled by per-core multiplicative mask streams prepared on the host.
"""

import numpy as np
import ml_dtypes

S = 4096
D = 1024
N_CORES = 8
P = 128
SH = 512              # per-core query shard rows / key window
N_WIN = 8
KT_ELEMS = D * SH           # per-rank kT block elements in AG buffer
V_ELEMS = SH * D
RANK_ELEMS = KT_ELEMS + V_ELEMS
AG_OUT_ELEMS = N_CORES * RANK_ELEMS
SCALE = 1.0 / 32.0          # 1/sqrt(D)

_CACHE = {}


def _build(win_mult=1, parts=frozenset({'sc','pv','act','msk','rs','dma','acc','ag'})):
    import concourse.bass as bass
    import concourse.mybir as mybir
    import concourse.tile as tile
    from concourse import bacc

    bf16 = mybir.dt.bfloat16
    f32 = mybir.dt.float32

    nc = bacc.Bacc("TRN2", target_bir_lowering=False, debug=False,
                   num_devices=N_CORES)

    # ---- per-core I/O ----
    wq = nc.dram_tensor("wq", [P, 8, D], bf16, kind="ExternalInput")
    wk = nc.dram_tensor("wk", [P, 8, D], bf16, kind="ExternalInput")
    wv = nc.dram_tensor("wv", [P, 8, D], bf16, kind="ExternalInput")
    xs = nc.dram_tensor("xs", [P, 8, SH], bf16, kind="ExternalInput")  # x^T shard
    maskd = nc.dram_tensor("mask", [N_WIN, 2, P, 2, SH], bf16, kind="ExternalInput")
    onesd = nc.dram_tensor("ones", [P, 1], bf16, kind="ExternalInput")
    outd = nc.dram_tensor("out", [SH, D], f32, kind="ExternalOutput")

    agin = nc.dram_tensor("agin", [1, RANK_ELEMS], bf16)
    agout = nc.dram_tensor("agout", [1, AG_OUT_ELEMS], bf16, addr_space="Shared")
    rs_dram = nc.dram_tensor("rs_dram", [1, SH], f32)

    with tile.TileContext(nc) as tc:
        with tc.tile_pool(name="wpool", bufs=3) as wpool, \
             tc.tile_pool(name="xpool", bufs=1) as xpool, \
             tc.tile_pool(name="qt", bufs=1) as qtpool, \
             tc.tile_pool(name="stage", bufs=3) as stage, \
             tc.tile_pool(name="consts", bufs=1) as consts, \
             tc.tile_pool(name="accs", bufs=1) as accs:

            # ---------------- Phase 1: projections ----------------
            xs_sb = xpool.tile([P, 8, SH], bf16, name="xs_sb")
            nc.sync.dma_start(xs_sb[:], xs[:])
            wk_sb = wpool.tile([P, 8, D], bf16, name="wk_sb", tag="w")
            nc.sync.dma_start(wk_sb[:], wk[:])
            wv_sb = wpool.tile([P, 8, D], bf16, name="wv_sb", tag="w")
            nc.sync.dma_start(wv_sb[:], wv[:])
            wq_sb = wpool.tile([P, 8, D], bf16, name="wq_sb", tag="w")
            nc.sync.dma_start(wq_sb[:], wq[:])

            with tc.tile_pool(name="pps", bufs=2, space="PSUM") as pps, \
                 tc.tile_pool(name="ppsv", bufs=2, space="PSUM") as ppsv:
                # kT_c: [8 dko][128 dp][512 j] into agin[0 : KT_ELEMS]
                for dt2 in range(4):          # two d-tiles per psum tile
                    ps = pps.tile([P, 2, SH], f32, name=f"kt_ps{dt2}", tag="ktps")
                    for h in range(2):
                        for ko in range(8):
                            d0 = (dt2 * 2 + h) * P
                            nc.tensor.matmul(ps[:, h, :],
                                             wk_sb[:, ko, d0:d0 + P],
                                             xs_sb[:, ko, :],
                                             start=(ko == 0), stop=(ko == 7))
                    st = stage.tile([P, 2, SH], bf16, name=f"kt_st{dt2}", tag="ktst")
                    nc.vector.tensor_copy(st[:], ps[:])
                    dst = bass.AP(agin, dt2 * (2 * P * SH),
                                  [[SH, P], [P * SH, 2], [1, SH]])
                    nc.sync.dma_start(dst, st[:])

                # v_c: [512 s][1024 d] into agin[KT_ELEMS : ]
                for st_i in range(4):
                    ps = ppsv.tile([P, D], f32, name=f"v_ps{st_i}", tag="vps")
                    for ko in range(8):
                        for dh in range(2):
                            nc.tensor.matmul(
                                ps[:, dh * 512:(dh + 1) * 512],
                                xs_sb[:, ko, st_i * P:(st_i + 1) * P],
                                wv_sb[:, ko, dh * 512:(dh + 1) * 512],
                                start=(ko == 0), stop=(ko == 7))
                    st = stage.tile([P, D], bf16, name=f"v_st{st_i}", tag="vst")
                    nc.vector.tensor_copy(st[:], ps[:])
                    dst = bass.AP(agin, KT_ELEMS + st_i * (P * D),
                                  [[D, P], [1, D]])
                    nc.sync.dma_start(dst, st[:])

                # ---------------- Phase 2: AllGather K/V ----------------
                if 'ag' in parts:
                    nc.gpsimd.collective_compute(
                        "AllGather", mybir.AluOpType.bypass,
                        replica_groups=[list(range(N_CORES))],
                        ins=[agin.ap().opt()],
                        outs=[agout.ap().opt()],
                    )
                else:
                    nc.sync.dma_start(
                        bass.AP(agout, 0, [[1, 1], [2048, RANK_ELEMS // 2048], [1, 2048]]),
                        bass.AP(agin, 0, [[1, 1], [2048, RANK_ELEMS // 2048], [1, 2048]]))

                # qT_c: keep in SBUF [128 dp, 8 dko, 512 i] (overlaps AG)
                qt_sb = qtpool.tile([P, 8, SH], bf16, name="qt_sb")
                for dt2 in range(4):
                    ps = pps.tile([P, 2, SH], f32, name=f"q_ps{dt2}", tag="ktps")
                    for h in range(2):
                        for ko in range(8):
                            d0 = (dt2 * 2 + h) * P
                            nc.tensor.matmul(ps[:, h, :],
                                             wq_sb[:, ko, d0:d0 + P],
                                             xs_sb[:, ko, :],
                                             start=(ko == 0), stop=(ko == 7))
                    nc.vector.tensor_copy(qt_sb[:, 2 * dt2:2 * dt2 + 2, :], ps[:])

            # ---------------- Phase 3: attention ----------------
            ones_sb = consts.tile([P, 1], bf16, name="ones_sb")
            nc.sync.dma_start(ones_sb[:], onesd[:])
            acc_out = accs.tile([P, 4, D], f32, name="acc_out")
            nc.vector.memset(acc_out[:], 0.0)

            with tc.tile_pool(name="ktw", bufs=2) as ktw, \
                 tc.tile_pool(name="vw", bufs=2) as vw, \
                 tc.tile_pool(name="mk", bufs=2) as mkp, \
                 tc.tile_pool(name="pt", bufs=3) as ptp, \
                 tc.tile_pool(name="stps", bufs=1, space="PSUM") as stps, \
                 tc.tile_pool(name="pvps", bufs=1, space="PSUM") as pvps, \
                 tc.tile_pool(name="rsps", bufs=1, space="PSUM") as rsps:

                rs_ps = rsps.tile([1, SH], f32, name="rs_ps")

                n_win_total = N_WIN * win_mult
                for wi in range(n_win_total):
                    W = wi % N_WIN
                    first = wi == 0
                    last = wi == n_win_total - 1
                    kt_w = ktw.tile([P, 8, SH], bf16, name=f"kt_w{wi}", tag="ktw")
                    nc.sync.dma_start(
                        kt_w[:], bass.AP(agout, W * RANK_ELEMS,
                                         [[SH, P], [P * SH, 8], [1, SH]]))
                    v_w = vw.tile([P, 4, D], bf16, name=f"v_w{wi}", tag="vw")
                    nc.sync.dma_start(
                        v_w[:], bass.AP(agout, W * RANK_ELEMS + KT_ELEMS,
                                        [[D, P], [P * D, 4], [1, D]]))

                    pv_ps = pvps.tile([P, 2, D], f32, name=f"pv{wi}", tag="pv")
                    pts = []
                    for pair in range(2):
                        st_ps = stps.tile([P, 2, SH], f32,
                                          name=f"st{wi}_{pair}", tag="st")
                        if 'sc' in parts:
                            for js in range(2):
                                jj = pair * 2 + js
                                for ko in range(8):
                                    nc.tensor.matmul(
                                        st_ps[:, js, :],
                                        kt_w[:, ko, jj * P:(jj + 1) * P],
                                        qt_sb[:, ko, :],
                                        start=(ko == 0), stop=(ko == 7))
                        else:
                            nc.vector.memset(st_ps[:], 0.1)
                        pt = ptp.tile([P, 2, SH], bf16, name=f"pt{wi}_{pair}",
                                      tag="pt")
                        if 'act' in parts:
                            nc.scalar.activation(pt[:], st_ps[:],
                                                 mybir.ActivationFunctionType.Exp,
                                                 scale=SCALE)
                        else:
                            nc.vector.tensor_copy(pt[:], st_ps[:])
                        if 'msk' in parts:
                            mk = mkp.tile([P, 2, SH], bf16, name=f"mk{wi}_{pair}",
                                          tag="mk")
                            nc.sync.dma_start(mk[:], maskd[W, pair])
                            nc.vector.tensor_mul(pt[:], pt[:], mk[:])
                        pts.append(pt)
                        if 'rs' in parts:
                            for js in range(2):
                                nc.tensor.matmul(
                                    rs_ps[:], ones_sb[:], pt[:, js, :],
                                    start=(first and pair == 0 and js == 0),
                                    stop=(last and pair == 1 and js == 1))

                    # PV: two isub-pair passes over all 4 j-subtiles
                    for half in range(2 if 'pv' in parts else 0):
                        for pair in range(2):
                            for js in range(2):
                                for ib in range(2):
                                    iblk = half * 2 + ib
                                    for dh in range(2):
                                        nc.tensor.matmul(
                                            pv_ps[:, ib, dh * 512:(dh + 1) * 512],
                                            pts[pair][:, js,
                                                      iblk * P:(iblk + 1) * P],
                                            v_w[:, pair * 2 + js,
                                                dh * 512:(dh + 1) * 512],
                                            start=(pair == 0 and js == 0),
                                            stop=(pair == 1 and js == 1))
                        for ib in range(2):
                            iblk = half * 2 + ib
                            nc.vector.tensor_add(acc_out[:, iblk, :],
                                                 acc_out[:, iblk, :],
                                                 pv_ps[:, ib, :])

                # ---------------- finalize: divide by row sums ----------------
                if 'rs' not in parts:
                    nc.vector.memset(rs_ps[:], 1.0)
                rs_sb = consts.tile([1, SH], f32, name="rs_sb")
                nc.vector.reciprocal(rs_sb[:], rs_ps[:])
                nc.sync.dma_start(rs_dram.ap(), rs_sb[:])
                recipT = consts.tile([P, 4], f32, name="recipT")
                nc.sync.dma_start(
                    recipT[:],
                    rs_dram.ap().rearrange("o (ib p) -> (o p) ib", p=P))
                o_f32 = accs.tile([P, 4, D], f32, name="o_f32")
                nc.vector.tensor_tensor(
                    o_f32[:], acc_out[:],
                    recipT[:, :, None].to_broadcast((P, 4, D)),
                    mybir.AluOpType.mult)
                nc.sync.dma_start(
                    outd.ap().rearrange("(ib p) d -> p ib d", p=P), o_f32[:])

    nc.compile()
    return nc


def _host_inputs(x, W_query, W_key, W_value):
    bf = ml_dtypes.bfloat16

    def wprep(W):
        return np.ascontiguousarray(
            W.reshape(8, P, D).transpose(1, 0, 2)).astype(bf)

    wq_n, wk_n, wv_n = wprep(W_query), wprep(W_key), wprep(W_value)

    in_maps = []
    for c in range(N_CORES):
        rows = np.arange(SH * c, SH * (c + 1))
        xt = x[rows].T.reshape(8, P, SH).transpose(1, 0, 2)
        xs_n = np.ascontiguousarray(xt).astype(bf)

        # mask[W, pair, p, js, i]: valid iff key (512W + (2*pair+js)*128 + p)
        #                               <= query (512c + i)
        mask = np.zeros((N_WIN, 2, P, 2, SH), dtype=np.float32)
        for Wn in range(N_WIN):
            if Wn < c:
                mask[Wn] = 1.0
            elif Wn == c:
                for pair in range(2):
                    for js in range(2):
                        jj = pair * 2 + js
                        j_rel = jj * P + np.arange(P)[:, None]
                        i_rel = np.arange(SH)[None, :]
                        mask[Wn, pair, :, js, :] = (j_rel <= i_rel)
        in_maps.append({
            "wq": wq_n, "wk": wk_n, "wv": wv_n, "xs": xs_n,
            "mask": mask.astype(bf),
            "ones": np.ones((P, 1), dtype=bf),
        })
    return in_maps


def kernel(x, W_query, W_key, W_value):
    from concourse.bass_utils import run_bass_kernel_spmd

    x = np.asarray(x, dtype=np.float32)
    W_query = np.asarray(W_query, dtype=np.float32)
    W_key = np.asarray(W_key, dtype=np.float32)
    W_value = np.asarray(W_value, dtype=np.float32)

    if "nc" not in _CACHE:
        _CACHE["nc"] = _build()
    nc = _CACHE["nc"]

    in_maps = _host_inputs(x, W_query, W_key, W_value)
    res = run_bass_kernel_spmd(nc, in_maps, core_ids=list(range(N_CORES)))

    out = np.empty((S, D), dtype=np.float32)
    for c in range(N_CORES):
        out[SH * c:SH * (c + 1)] = res.results[c]["out"]
    return out


# revision 16
# speedup vs baseline: 9918.1247x; 1.1772x over previous
"""Causal self-attention (single head, S=4096, D=1024) on 8 TRN2 NeuronCores.

Strategy (sequence-parallel, instruction-count-optimized):
  - Core c owns contiguous query rows [512c, 512(c+1)) and computes the
    K/V projections for the same rows; K^T/V are AllGathered in bf16.
  - Scores are computed TRANSPOSED (S^T[j, i], keys on partitions): softmax
    needs no max-subtraction (scores are O(5)), exp(S^T) feeds the PV matmul
    directly as lhsT (no transposes), row sums come from a ones-vector matmul
    accumulated in PSUM across the whole loop.
  - Every core runs the same fully-static program over all 8 key windows
    (window W = AG rank block W); causality and the padded (future) windows
    are handled by per-core multiplicative mask streams prepared on the host.
"""

import numpy as np
import ml_dtypes

S = 4096
D = 1024
N_CORES = 8
P = 128
SH = 512              # per-core query shard rows / key window
N_WIN = 8
KT_ELEMS = D * SH           # per-rank kT block elements in AG buffer
V_ELEMS = SH * D
RANK_ELEMS = KT_ELEMS + V_ELEMS
AG_OUT_ELEMS = N_CORES * RANK_ELEMS
SCALE = 1.0 / 32.0          # 1/sqrt(D)

_CACHE = {}


def _build(win_mult=1, pv_split=True, st_split=True, dma_split=2, wbufs=2, parts=frozenset({'sc','pv','act','msk','rs','dma','acc','ag'})):
    import concourse.bass as bass
    import concourse.mybir as mybir
    import concourse.tile as tile
    from concourse import bacc

    bf16 = mybir.dt.bfloat16
    f32 = mybir.dt.float32

    nc = bacc.Bacc("TRN2", target_bir_lowering=False, debug=False,
                   num_devices=N_CORES)

    # ---- per-core I/O ----
    wq = nc.dram_tensor("wq", [P, 8, D], bf16, kind="ExternalInput")
    wk = nc.dram_tensor("wk", [P, 8, D], bf16, kind="ExternalInput")
    wv = nc.dram_tensor("wv", [P, 8, D], bf16, kind="ExternalInput")
    xs = nc.dram_tensor("xs", [P, 8, SH], bf16, kind="ExternalInput")  # x^T shard
    maskd = nc.dram_tensor("mask", [N_WIN, 2, P, 2, SH], bf16, kind="ExternalInput")
    onesd = nc.dram_tensor("ones", [P, 1], bf16, kind="ExternalInput")
    outd = nc.dram_tensor("out", [SH, D], f32, kind="ExternalOutput")

    agin = nc.dram_tensor("agin", [1, RANK_ELEMS], bf16)
    agout = nc.dram_tensor("agout", [1, AG_OUT_ELEMS], bf16, addr_space="Shared")
    rs_dram = nc.dram_tensor("rs_dram", [1, SH], f32)

    with tile.TileContext(nc) as tc:
        with tc.tile_pool(name="wpool", bufs=3) as wpool, \
             tc.tile_pool(name="xpool", bufs=1) as xpool, \
             tc.tile_pool(name="qt", bufs=1) as qtpool, \
             tc.tile_pool(name="stage", bufs=3) as stage, \
             tc.tile_pool(name="consts", bufs=1) as consts, \
             tc.tile_pool(name="accs", bufs=1) as accs:

            # ---------------- Phase 1: projections ----------------
            xs_sb = xpool.tile([P, 8, SH], bf16, name="xs_sb")
            wk_sb = wpool.tile([P, 8, D], bf16, name="wk_sb", tag="w")
            wv_sb = wpool.tile([P, 8, D], bf16, name="wv_sb", tag="w")
            wq_sb = wpool.tile([P, 8, D], bf16, name="wq_sb", tag="w")
            for ko in range(8):
                nc.sync.dma_start(xs_sb[:, ko, :], xs[:, ko, :])
                nc.sync.dma_start(wk_sb[:, ko, :], wk[:, ko, :])
            for ko in range(8):
                nc.sync.dma_start(wv_sb[:, ko, :], wv[:, ko, :])
                nc.sync.dma_start(wq_sb[:, ko, :], wq[:, ko, :])

            with tc.tile_pool(name="pps", bufs=2, space="PSUM") as pps, \
                 tc.tile_pool(name="ppsv", bufs=2, space="PSUM") as ppsv:
                # kT_c: [8 dko][128 dp][512 j] into agin[0 : KT_ELEMS]
                for dt2 in range(4):          # two d-tiles per psum tile
                    ps = pps.tile([P, 2, SH], f32, name=f"kt_ps{dt2}", tag="ktps")
                    for h in range(2):
                        for ko in range(8):
                            d0 = (dt2 * 2 + h) * P
                            nc.tensor.matmul(ps[:, h, :],
                                             wk_sb[:, ko, d0:d0 + P],
                                             xs_sb[:, ko, :],
                                             start=(ko == 0), stop=(ko == 7))
                    st = stage.tile([P, 2, SH], bf16, name=f"kt_st{dt2}", tag="ktst")
                    nc.vector.tensor_copy(st[:], ps[:])
                    dst = bass.AP(agin, dt2 * (2 * P * SH),
                                  [[SH, P], [P * SH, 2], [1, SH]])
                    nc.sync.dma_start(dst, st[:])

                # v_c: [512 s][1024 d] into agin[KT_ELEMS : ]
                for st_i in range(4):
                    ps = ppsv.tile([P, D], f32, name=f"v_ps{st_i}", tag="vps")
                    for ko in range(8):
                        for dh in range(2):
                            nc.tensor.matmul(
                                ps[:, dh * 512:(dh + 1) * 512],
                                xs_sb[:, ko, st_i * P:(st_i + 1) * P],
                                wv_sb[:, ko, dh * 512:(dh + 1) * 512],
                                start=(ko == 0), stop=(ko == 7))
                    st = stage.tile([P, D], bf16, name=f"v_st{st_i}", tag="vst")
                    nc.vector.tensor_copy(st[:], ps[:])
                    dst = bass.AP(agin, KT_ELEMS + st_i * (P * D),
                                  [[D, P], [1, D]])
                    nc.sync.dma_start(dst, st[:])

                # ---------------- Phase 2: AllGather K/V ----------------
                if 'ag' in parts:
                    nc.gpsimd.collective_compute(
                        "AllGather", mybir.AluOpType.bypass,
                        replica_groups=[list(range(N_CORES))],
                        ins=[agin.ap().opt()],
                        outs=[agout.ap().opt()],
                    )
                else:
                    for sp8 in range(8):
                        off8 = sp8 * (RANK_ELEMS // 8)
                        nc.sync.dma_start(
                            bass.AP(agout, off8, [[1, 1], [2048, RANK_ELEMS // 16384], [1, 2048]]),
                            bass.AP(agin, off8, [[1, 1], [2048, RANK_ELEMS // 16384], [1, 2048]]))

                # qT_c: keep in SBUF [128 dp, 8 dko, 512 i] (overlaps AG)
                qt_sb = qtpool.tile([P, 8, SH], bf16, name="qt_sb")
                for dt2 in range(4):
                    ps = pps.tile([P, 2, SH], f32, name=f"q_ps{dt2}", tag="ktps")
                    for h in range(2):
                        for ko in range(8):
                            d0 = (dt2 * 2 + h) * P
                            nc.tensor.matmul(ps[:, h, :],
                                             wq_sb[:, ko, d0:d0 + P],
                                             xs_sb[:, ko, :],
                                             start=(ko == 0), stop=(ko == 7))
                    nc.vector.tensor_copy(qt_sb[:, 2 * dt2:2 * dt2 + 2, :], ps[:])

            # ---------------- Phase 3: attention ----------------
            ones_sb = consts.tile([P, 1], bf16, name="ones_sb")
            nc.sync.dma_start(ones_sb[:], onesd[:])
            acc_out = accs.tile([P, 4, D], f32, name="acc_out")
            nc.vector.memset(acc_out[:], 0.0)

            with tc.tile_pool(name="ktw", bufs=wbufs) as ktw, \
                 tc.tile_pool(name="vw", bufs=wbufs) as vw, \
                 tc.tile_pool(name="mk", bufs=2) as mkp, \
                 tc.tile_pool(name="pt", bufs=3) as ptp, \
                 tc.tile_pool(name="stps", bufs=1, space="PSUM") as stps, \
                 tc.tile_pool(name="pvps", bufs=1, space="PSUM") as pvps, \
                 tc.tile_pool(name="rsps", bufs=1, space="PSUM") as rsps:

                rs_ps = rsps.tile([1, SH], f32, name="rs_ps")

                n_win_total = N_WIN * win_mult
                for wi in range(n_win_total):
                    W = wi % N_WIN
                    first = wi == 0
                    last = wi == n_win_total - 1
                    kt_w = ktw.tile([P, 8, SH], bf16, name=f"kt_w{wi}", tag="ktw")
                    v_w = vw.tile([P, 4, D], bf16, name=f"v_w{wi}", tag="vw")
                    ns = dma_split
                    for sp in range(ns):
                        ko0, kon = sp * (8 // ns), 8 // ns
                        nc.sync.dma_start(
                            kt_w[:, ko0:ko0 + kon, :],
                            bass.AP(agout, W * RANK_ELEMS + ko0 * P * SH,
                                    [[SH, P], [P * SH, kon], [1, SH]]))
                        jo0, jon = sp * (4 // ns), 4 // ns
                        nc.sync.dma_start(
                            v_w[:, jo0:jo0 + jon, :],
                            bass.AP(agout,
                                    W * RANK_ELEMS + KT_ELEMS + jo0 * P * D,
                                    [[D, P], [P * D, jon], [1, D]]))

                    if not pv_split:
                        pv_ps = pvps.tile([P, 2, D], f32, name=f"pv{wi}", tag="pv")
                    pts = []
                    for pair in range(2):
                        if st_split:
                            st_ps = stps.tile([P, 2, SH], f32,
                                              name=f"st{wi}_{pair}", tag="st",
                                              bufs=2)
                        else:
                            st_ps = stps.tile([P, 2, SH], f32,
                                              name=f"st{wi}_{pair}", tag="st")
                        if 'sc' in parts:
                            for js in range(2):
                                jj = pair * 2 + js
                                for ko in range(8):
                                    nc.tensor.matmul(
                                        st_ps[:, js, :],
                                        kt_w[:, ko, jj * P:(jj + 1) * P],
                                        qt_sb[:, ko, :],
                                        start=(ko == 0), stop=(ko == 7))
                        else:
                            nc.vector.memset(st_ps[:], 0.1)
                        pt = ptp.tile([P, 2, SH], bf16, name=f"pt{wi}_{pair}",
                                      tag="pt")
                        if 'act' in parts:
                            nc.scalar.activation(pt[:], st_ps[:],
                                                 mybir.ActivationFunctionType.Exp,
                                                 scale=SCALE)
                        else:
                            nc.vector.tensor_copy(pt[:], st_ps[:])
                        if 'msk' in parts:
                            mk = mkp.tile([P, 2, SH], bf16, name=f"mk{wi}_{pair}",
                                          tag="mk")
                            nc.sync.dma_start(mk[:], maskd[W, pair])
                            nc.vector.tensor_mul(pt[:], pt[:], mk[:])
                        pts.append(pt)
                        if 'rs' in parts:
                            for js in range(2):
                                nc.tensor.matmul(
                                    rs_ps[:], ones_sb[:], pt[:, js, :],
                                    start=(first and pair == 0 and js == 0),
                                    stop=(last and pair == 1 and js == 1))

                    # PV: isub-pair (or per-iblk if pv_split) passes
                    if pv_split and 'pv' in parts:
                        for iblk in range(4):
                            pvq = pvps.tile([P, 1, D], f32,
                                            name=f"pvq{wi}_{iblk}", tag="pv")
                            for pair in range(2):
                                for js in range(2):
                                    for dh in range(2):
                                        nc.tensor.matmul(
                                            pvq[:, 0, dh * 512:(dh + 1) * 512],
                                            pts[pair][:, js,
                                                      iblk * P:(iblk + 1) * P],
                                            v_w[:, pair * 2 + js,
                                                dh * 512:(dh + 1) * 512],
                                            start=(pair == 0 and js == 0),
                                            stop=(pair == 1 and js == 1))
                            nc.vector.tensor_add(acc_out[:, iblk, :],
                                                 acc_out[:, iblk, :],
                                                 pvq[:, 0, :])
                    else:
                      for half in range(2 if 'pv' in parts else 0):
                        for pair in range(2):
                            for js in range(2):
                                for ib in range(2):
                                    iblk = half * 2 + ib
                                    for dh in range(2):
                                        nc.tensor.matmul(
                                            pv_ps[:, ib, dh * 512:(dh + 1) * 512],
                                            pts[pair][:, js,
                                                      iblk * P:(iblk + 1) * P],
                                            v_w[:, pair * 2 + js,
                                                dh * 512:(dh + 1) * 512],
                                            start=(pair == 0 and js == 0),
                                            stop=(pair == 1 and js == 1))
                        for ib in range(2):
                            iblk = half * 2 + ib
                            nc.vector.tensor_add(acc_out[:, iblk, :],
                                                 acc_out[:, iblk, :],
                                                 pv_ps[:, ib, :])

                # ---------------- finalize: divide by row sums ----------------
                if 'rs' not in parts:
                    nc.vector.memset(rs_ps[:], 1.0)
                rs_sb = consts.tile([1, SH], f32, name="rs_sb")
                nc.vector.reciprocal(rs_sb[:], rs_ps[:])
                nc.sync.dma_start(rs_dram.ap(), rs_sb[:])
                recipT = consts.tile([P, 4], f32, name="recipT")
                nc.sync.dma_start(
                    recipT[:],
                    rs_dram.ap().rearrange("o (ib p) -> (o p) ib", p=P))
                o_f32 = accs.tile([P, 4, D], f32, name="o_f32")
                nc.vector.tensor_tensor(
                    o_f32[:], acc_out[:],
                    recipT[:, :, None].to_broadcast((P, 4, D)),
                    mybir.AluOpType.mult)
                nc.sync.dma_start(
                    outd.ap().rearrange("(ib p) d -> p ib d", p=P), o_f32[:])

    nc.compile()
    return nc


def _host_inputs(x, W_query, W_key, W_value):
    bf = ml_dtypes.bfloat16

    def wprep(W):
        return np.ascontiguousarray(
            W.reshape(8, P, D).transpose(1, 0, 2)).astype(bf)

    wq_n, wk_n, wv_n = wprep(W_query), wprep(W_key), wprep(W_value)

    in_maps = []
    for c in range(N_CORES):
        rows = np.arange(SH * c, SH * (c + 1))
        xt = x[rows].T.reshape(8, P, SH).transpose(1, 0, 2)
        xs_n = np.ascontiguousarray(xt).astype(bf)

        # mask[W, pair, p, js, i]: valid iff key (512W + (2*pair+js)*128 + p)
        #                               <= query (512c + i)
        mask = np.zeros((N_WIN, 2, P, 2, SH), dtype=np.float32)
        for Wn in range(N_WIN):
            if Wn < c:
                mask[Wn] = 1.0
            elif Wn == c:
                for pair in range(2):
                    for js in range(2):
                        jj = pair * 2 + js
                        j_rel = jj * P + np.arange(P)[:, None]
                        i_rel = np.arange(SH)[None, :]
                        mask[Wn, pair, :, js, :] = (j_rel <= i_rel)
        in_maps.append({
            "wq": wq_n, "wk": wk_n, "wv": wv_n, "xs": xs_n,
            "mask": mask.astype(bf),
            "ones": np.ones((P, 1), dtype=bf),
        })
    return in_maps


def kernel(x, W_query, W_key, W_value):
    from concourse.bass_utils import run_bass_kernel_spmd

    x = np.asarray(x, dtype=np.float32)
    W_query = np.asarray(W_query, dtype=np.float32)
    W_key = np.asarray(W_key, dtype=np.float32)
    W_value = np.asarray(W_value, dtype=np.float32)

    if "nc" not in _CACHE:
        _CACHE["nc"] = _build()
    nc = _CACHE["nc"]

    in_maps = _host_inputs(x, W_query, W_key, W_value)
    res = run_bass_kernel_spmd(nc, in_maps, core_ids=list(range(N_CORES)))

    out = np.empty((S, D), dtype=np.float32)
    for c in range(N_CORES):
        out[SH * c:SH * (c + 1)] = res.results[c]["out"]
    return out
